# revision 16
# baseline (speedup 1.0000x reference)
"""Causal MHA + RoPE kernel for 8 trn2 NeuronCores.

Sharding: core id = (b, hh) with b = id//2 (batch), hh = id%2 (head half).
Launch 1: each core computes qkv projection + RoPE + causal attention for
its 8 heads over the full sequence of its batch element, writing the
(normalized) attention output slice ao [512 c, 2048 t] (channel-major).
Host: stacks the two head-half slices per batch -> aoT [1024 c, 2048 t].
Launch 2: core id = (b, th): output projection for a 1024-row t-slab.
All matmuls run in float32r (full PE rate at free-dim >= 256).
"""

import numpy as np
import concourse.bass as bass
import concourse.bacc as bacc
import concourse.mybir as mybir
from concourse import tile
from concourse.bass_utils import run_bass_kernel_spmd

B, T, D, H, HD = 4, 2048, 1024, 16, 64
F32 = mybir.dt.float32
F32R = mybir.dt.float32r
NCORE = 8
NG = 4          # head-pair groups per core (8 heads / 2)
TCH = 512       # t-chunk width in projection
NKB = T // 128  # 16 k-blocks
NQB = T // 512  # 4 q-blocks
# head-dim permutation putting rotate-half pairs 16 partitions apart
PERM = (list(range(0, 16)) + list(range(32, 48))
        + list(range(16, 32)) + list(range(48, 64)))
SHUF_MASK = [(i + 16) % 32 for i in range(32)]


def build_launch1():
    nc = bacc.Bacc("TRN2", target_bir_lowering=False, debug=False,
                   num_devices=NCORE)
    xt = nc.dram_tensor("xt", [D, T], F32R, kind="ExternalInput")
    wq = nc.dram_tensor("wq", [D, 512], F32R, kind="ExternalInput")
    wk = nc.dram_tensor("wk", [D, 512], F32R, kind="ExternalInput")
    wv = nc.dram_tensor("wv", [D, 512], F32R, kind="ExternalInput")
    ct = nc.dram_tensor("ct", [128, T], F32, kind="ExternalInput")
    st = nc.dram_tensor("st", [128, T], F32, kind="ExternalInput")
    tri = nc.dram_tensor("tri", [128, 128], F32R, kind="ExternalInput")
    vones = nc.dram_tensor("vones", [128, 8], F32R, kind="ExternalInput")
    ao = nc.dram_tensor("ao", [512, T], F32R, kind="ExternalOutput")

    with tile.TileContext(nc) as tc:
        with (
            tc.tile_pool(name="const", bufs=1) as cpool,
            tc.tile_pool(name="wsb", bufs=1) as wpool,
            tc.tile_pool(name="xtc", bufs=1) as xpool,
            tc.tile_pool(name="vaug", bufs=1) as vpool,
            tc.tile_pool(name="qkt", bufs=1) as qkpool,
            tc.tile_pool(name="rope", bufs=1) as rpool,
            tc.tile_pool(name="pt", bufs=2) as ppool,
            tc.tile_pool(name="norm", bufs=1) as npool,
            tc.tile_pool(name="aot", bufs=2) as aopool,
            tc.tile_pool(name="proj", bufs=2, space="PSUM") as projps,
            tc.tile_pool(name="score", bufs=2, space="PSUM") as scoreps,
            tc.tile_pool(name="outt", bufs=2, space="PSUM") as outps,
        ):
            ct_sb = cpool.tile([128, T], F32, tag="ct")
            st_sb = cpool.tile([128, T], F32, tag="st")
            vones_sb = cpool.tile([128, 8], F32R, tag="vones")
            nc.sync.dma_start(vones_sb[:], vones[:])
            tri_sb = cpool.tile([128, 128], F32R, tag="tri")
            nc.sync.dma_start(ct_sb[:], ct[:])
            nc.sync.dma_start(st_sb[:], st[:])
            nc.sync.dma_start(tri_sb[:], tri[:])

            wq_sb = wpool.tile([128, 8, 512], F32R, tag="wq")
            wk_sb = wpool.tile([128, 8, 512], F32R, tag="wk")
            wv_sb = wpool.tile([128, 8, 512], F32R, tag="wv")
            for wsb, wdr in ((wq_sb, wq), (wk_sb, wk), (wv_sb, wv)):
                nc.sync.dma_start(
                    wsb[:], wdr.rearrange("(a p) f -> p a f", p=128))

            # v_aug tiles: [128 t, 8 heads * 65]; col 65h+64 is the ones
            # column used to accumulate softmax denominators during AV.
            v_aug = [vpool.tile([128, 8 * 65], F32R, tag=f"va{i}", name=f"va{i}")
                     for i in range(NKB)]
            for i in range(NKB):
                va = v_aug[i].rearrange("p (h w) -> p h w", w=65)
                nc.sync.dma_start(
                    va[:, :, 64:65],
                    vones.rearrange("p (h w) -> p h w", w=1))

            qt_sb = [qkpool.tile([128, T], F32R, tag=f"qt{g}", name=f"qt{g}")
                     for g in range(NG)]
            kt_sb = [qkpool.tile([128, T], F32R, tag=f"kt{g}", name=f"kt{g}")
                     for g in range(NG)]

            # ---------------- projection + RoPE ----------------
            for tch in range(T // TCH):
                t0 = tch * TCH
                xtc = xpool.tile([128, 8, TCH], F32R, tag="xt")
                nc.sync.dma_start(
                    xtc[:],
                    xt.rearrange("(a p) t -> p a t", p=128)[:, :, t0:t0 + TCH])

                # v projection: out[t, f] for all 8 heads
                for ts in range(TCH // 128):
                    vps = projps.tile([128, 512], F32, tag="proj")
                    for a in range(8):
                        nc.tensor.matmul(
                            vps[:], xtc[:, a, ts * 128:(ts + 1) * 128],
                            wv_sb[:, a, :], start=(a == 0), stop=(a == 7))
                    va = v_aug[tch * 4 + ts].rearrange(
                        "p (h w) -> p h w", w=65)
                    nc.vector.tensor_copy(
                        va[:, :, 0:64],
                        vps.rearrange("p (h d) -> p h d", d=64))

                # q/k projections per head-pair group + RoPE
                for g in range(NG):
                    for kind, wsb, dst in (("q", wq_sb, qt_sb[g]),
                                           ("k", wk_sb, kt_sb[g])):
                        ps = projps.tile([128, TCH], F32, tag="proj")
                        for a in range(8):
                            nc.tensor.matmul(
                                ps[:], wsb[:, a, g * 128:(g + 1) * 128],
                                xtc[:, a, :], start=(a == 0), stop=(a == 7))
                        # RoPE in permuted-d layout: rotate-half pairs sit
                        # 16 partitions apart within a 32-quadrant, so
                        # stream_shuffle provides the rotated operand and
                        # all ops are full-width contiguous.
                        rq = rpool.tile([128, TCH], F32, tag="rq")
                        t1 = rpool.tile([128, TCH], F32, tag="t1")
                        t2 = rpool.tile([128, TCH], F32, tag="t2")
                        nc.vector.stream_shuffle(rq[:], ps[:], SHUF_MASK)
                        nc.vector.tensor_mul(
                            t1[:], ps[:], ct_sb[:, t0:t0 + TCH])
                        nc.vector.tensor_mul(
                            t2[:], rq[:], st_sb[:, t0:t0 + TCH])
                        nc.vector.tensor_add(
                            dst[:, t0:t0 + TCH], t1[:], t2[:])

            # ---------------- attention ----------------
            for g in range(NG):
                for qb in range(NQB):
                    q0 = qb * 512
                    nkb = 4 * (qb + 1)
                    out_t = [outps.tile([65, 512], F32, tag="outt", name=f"ot{g}_{qb}_{i}")
                             for i in range(2)]
                    for kbp in range(nkb // 2):
                        sc = [scoreps.tile([128, 1024], F32, tag="score", name=f"sc{g}_{qb}_{kbp}_{i}")
                              for i in range(2)]
                        pt = [ppool.tile([128, 1024], F32R, tag="pt", name=f"pt{g}_{qb}_{kbp}_{i}")
                              for i in range(2)]
                        for half in range(2):
                            kb = kbp * 2 + half
                            for h in range(2):
                                nc.tensor.matmul(
                                    sc[h][:, half * 512:(half + 1) * 512],
                                    kt_sb[g][h * 64:(h + 1) * 64,
                                             kb * 128:(kb + 1) * 128],
                                    qt_sb[g][h * 64:(h + 1) * 64,
                                             q0:q0 + 512],
                                    start=True, stop=True)
                        for half in range(2):
                            kb = kbp * 2 + half
                            off = max(0, (kb - 4 * qb)) * 128
                            co = half * 512
                            for h in range(2):
                                nc.scalar.activation(
                                    pt[h][:, co + off:co + 512],
                                    sc[h][:, co + off:co + 512],
                                    mybir.ActivationFunctionType.Exp,
                                    scale=0.125)
                                if kb >= 4 * qb:
                                    nc.vector.tensor_mul(
                                        pt[h][:, co + off:co + off + 128],
                                        pt[h][:, co + off:co + off + 128],
                                        tri_sb[:])
                                gh = 2 * g + h
                                nc.tensor.matmul(
                                    out_t[h][:, off:512],
                                    v_aug[kb][:, 65 * gh:65 * gh + 65],
                                    pt[h][:, co + off:co + 512],
                                    start=(kb == 0), stop=(kb == nkb - 1),
                                    skip_group_check=True)
                    for h in range(2):
                        gh = 2 * g + h
                        rec = npool.tile([1, 512], F32, tag="rec")
                        bc = npool.tile([64, 512], F32, tag="bc")
                        aot = aopool.tile([64, 512], F32R, tag="aot")
                        nc.vector.reciprocal(rec[:], out_t[h][64:65, :])
                        nc.gpsimd.partition_broadcast(bc[:], rec[:])
                        nc.vector.tensor_mul(aot[:], out_t[h][0:64, :], bc[:])
                        nc.sync.dma_start(
                            ao[gh * 64:(gh + 1) * 64, q0:q0 + 512], aot[:])
    nc.compile()
    return nc


def build_launch2():
    nc = bacc.Bacc("TRN2", target_bir_lowering=False, debug=False,
                   num_devices=NCORE)
    ao2 = nc.dram_tensor("ao2", [D, 1024], F32R, kind="ExternalInput")
    wo = nc.dram_tensor("wo", [D, D], F32R, kind="ExternalInput")
    y = nc.dram_tensor("y", [D, 1024], F32, kind="ExternalOutput")

    with tile.TileContext(nc) as tc:
        with (
            tc.tile_pool(name="asb", bufs=1) as apool,
            tc.tile_pool(name="wsb", bufs=2) as wpool,
            tc.tile_pool(name="ysb", bufs=3) as ypool,
            tc.tile_pool(name="ps", bufs=4, space="PSUM") as psp,
        ):
            a_sb = apool.tile([128, 8, 1024], F32R, tag="a")
            nc.sync.dma_start(
                a_sb[:], ao2.rearrange("(a p) t -> p a t", p=128))
            for ot in range(8):
                wt = wpool.tile([128, 8, 128], F32R, tag="w")
                nc.sync.dma_start(
                    wt[:], wo.rearrange("(a p) o -> p a o", p=128)
                    [:, :, ot * 128:(ot + 1) * 128])
                for tt in range(2):
                    ps = psp.tile([128, 512], F32, tag="ps")
                    for a in range(8):
                        nc.tensor.matmul(
                            ps[:], wt[:, a, :],
                            a_sb[:, a, tt * 512:(tt + 1) * 512],
                            start=(a == 0), stop=(a == 7))
                    ysb = ypool.tile([128, 512], F32, tag="y")
                    nc.vector.tensor_copy(ysb[:], ps[:])
                    nc.sync.dma_start(
                        y[ot * 128:(ot + 1) * 128,
                          tt * 512:(tt + 1) * 512], ysb[:])
    nc.compile()
    return nc


_NC1 = None
_NC2 = None
LAST_EXEC1_NS = None
LAST_EXEC2_NS = None


def kernel(x, cos, sin, w_qkv, w_out):
    global _NC1, _NC2
    x = np.asarray(x, dtype=np.float32)
    cos = np.asarray(cos, dtype=np.float32)
    sin = np.asarray(sin, dtype=np.float32)
    w_qkv = np.asarray(w_qkv, dtype=np.float32)
    w_out = np.asarray(w_out, dtype=np.float32)
    if _NC1 is None:
        _NC1 = build_launch1()
        _NC2 = build_launch2()

    perm = np.asarray(PERM)
    sign = np.where(perm < 32, -1.0, 1.0).astype(np.float32)
    ctp = cos.T[perm]                          # [64, T] permuted-d cos
    stp = sin.T[perm] * sign[:, None]          # signed, permuted sin
    ctm = np.ascontiguousarray(np.vstack([ctp, ctp]))  # [128, T] head pair
    stm = np.ascontiguousarray(np.vstack([stp, stp]))
    trim = np.ascontiguousarray(
        (np.arange(128)[:, None] <= np.arange(128)[None, :])
        .astype(np.float32))                   # tri[k, q] = k <= q
    wqm = w_qkv[0:D]                           # [f, c] head-major
    wkm = w_qkv[D:2 * D]
    wvm = w_qkv[2 * D:3 * D]
    # permute rotate-half pairs within each head's 64 q/k channels
    wqp = wqm.reshape(H, HD, D)[:, perm].reshape(D, D)
    wkp = wkm.reshape(H, HD, D)[:, perm].reshape(D, D)
    ones8 = np.ones((128, 8), np.float32)

    ins1 = []
    for core in range(NCORE):
        b, hh = core // 2, core % 2
        r0 = hh * 512
        ins1.append({
            "xt": np.ascontiguousarray(x[b].T),
            "wq": np.ascontiguousarray(wqp[r0:r0 + 512].T),
            "wk": np.ascontiguousarray(wkp[r0:r0 + 512].T),
            "wv": np.ascontiguousarray(wvm[r0:r0 + 512].T),
            "ct": ctm, "st": stm, "tri": trim, "vones": ones8,
        })
    res1 = run_bass_kernel_spmd(_NC1, ins1, core_ids=list(range(NCORE)))
    global LAST_EXEC1_NS, LAST_EXEC2_NS
    LAST_EXEC1_NS = res1.exec_time_ns

    wom = np.ascontiguousarray(w_out.T)        # [c, o]
    ins2 = []
    for core in range(NCORE):
        b, th = core // 2, core % 2
        aot = np.vstack([res1.results[2 * b]["ao"],
                         res1.results[2 * b + 1]["ao"]])  # [1024 c, 2048 t]
        ins2.append({
            "ao2": np.ascontiguousarray(aot[:, th * 1024:(th + 1) * 1024]),
            "wo": wom,
        })
    res2 = run_bass_kernel_spmd(_NC2, ins2, core_ids=list(range(NCORE)))
    LAST_EXEC2_NS = res2.exec_time_ns

    out = np.empty((B, T, D), dtype=np.float32)
    for core in range(NCORE):
        b, th = core // 2, core % 2
        out[b, th * 1024:(th + 1) * 1024, :] = res2.results[core]["y"].T
    return out


# revision 20
# speedup vs baseline: 1.0065x; 1.0065x over previous
"""Causal MHA + RoPE kernel for 8 trn2 NeuronCores.

Sharding: core id = (b, hh) with b = id//2 (batch), hh = id%2 (head half).
Launch 1: each core computes qkv projection + RoPE + causal attention for
its 8 heads over the full sequence of its batch element, writing the
(normalized) attention output slice ao [512 c, 2048 t] (channel-major).
Host: stacks the two head-half slices per batch -> aoT [1024 c, 2048 t].
Launch 2: core id = (b, th): output projection for a 1024-row t-slab.
All matmuls run in float32r (full PE rate at free-dim >= 256).
"""

import numpy as np
import concourse.bass as bass
import concourse.bacc as bacc
import concourse.mybir as mybir
from concourse import tile
from concourse.bass_utils import run_bass_kernel_spmd

B, T, D, H, HD = 4, 2048, 1024, 16, 64
F32 = mybir.dt.float32
F32R = mybir.dt.float32r
NCORE = 8
NG = 4          # head-pair groups per core (8 heads / 2)
TCH = 512       # t-chunk width in projection
NKB = T // 128  # 16 k-blocks
NQB = T // 512  # 4 q-blocks
# head-dim permutation putting rotate-half pairs 16 partitions apart
PERM = (list(range(0, 16)) + list(range(32, 48))
        + list(range(16, 32)) + list(range(48, 64)))
SHUF_MASK = [(i + 16) % 32 for i in range(32)]


def build_launch1():
    nc = bacc.Bacc("TRN2", target_bir_lowering=False, debug=False,
                   num_devices=NCORE)
    xt = nc.dram_tensor("xt", [D, T], F32R, kind="ExternalInput")
    wq = nc.dram_tensor("wq", [D, 512], F32R, kind="ExternalInput")
    wk = nc.dram_tensor("wk", [D, 512], F32R, kind="ExternalInput")
    wv = nc.dram_tensor("wv", [D, 512], F32R, kind="ExternalInput")
    ct = nc.dram_tensor("ct", [128, T], F32, kind="ExternalInput")
    st = nc.dram_tensor("st", [128, T], F32, kind="ExternalInput")
    tri = nc.dram_tensor("tri", [128, 128], F32R, kind="ExternalInput")
    vones = nc.dram_tensor("vones", [128, 8], F32R, kind="ExternalInput")
    ao = nc.dram_tensor("ao", [512, T], F32R, kind="ExternalOutput")

    with tile.TileContext(nc) as tc:
        with (
            tc.tile_pool(name="const", bufs=1) as cpool,
            tc.tile_pool(name="wsb", bufs=1) as wpool,
            tc.tile_pool(name="xtc", bufs=1) as xpool,
            tc.tile_pool(name="vaug", bufs=1) as vpool,
            tc.tile_pool(name="qkt", bufs=1) as qkpool,
            tc.tile_pool(name="rope", bufs=1) as rpool,
            tc.tile_pool(name="pt", bufs=2) as ppool,
            tc.tile_pool(name="norm", bufs=1) as npool,
            tc.tile_pool(name="aot", bufs=2) as aopool,
            tc.tile_pool(name="proj", bufs=2, space="PSUM") as projps,
            tc.tile_pool(name="score", bufs=2, space="PSUM") as scoreps,
            tc.tile_pool(name="outt", bufs=2, space="PSUM") as outps,
        ):
            ct_sb = cpool.tile([128, T], F32, tag="ct")
            st_sb = cpool.tile([128, T], F32, tag="st")
            vones_sb = cpool.tile([128, 8], F32R, tag="vones")
            nc.sync.dma_start(vones_sb[:], vones[:])
            tri_sb = cpool.tile([128, 128], F32R, tag="tri")
            nc.sync.dma_start(ct_sb[:], ct[:])
            nc.sync.dma_start(st_sb[:], st[:])
            nc.sync.dma_start(tri_sb[:], tri[:])

            wq_sb = wpool.tile([128, 8, 512], F32R, tag="wq")
            wk_sb = wpool.tile([128, 8, 512], F32R, tag="wk")
            wv_sb = wpool.tile([128, 8, 512], F32R, tag="wv")
            for wsb, wdr in ((wq_sb, wq), (wk_sb, wk), (wv_sb, wv)):
                nc.sync.dma_start(
                    wsb[:], wdr.rearrange("(a p) f -> p a f", p=128))

            # v_aug tiles: [128 t, 8 heads * 65]; col 65h+64 is the ones
            # column used to accumulate softmax denominators during AV.
            v_aug = [vpool.tile([128, 8 * 65], F32R, tag=f"va{i}", name=f"va{i}")
                     for i in range(NKB)]
            for i in range(NKB):
                va = v_aug[i].rearrange("p (h w) -> p h w", w=65)
                nc.sync.dma_start(
                    va[:, :, 64:65],
                    vones.rearrange("p (h w) -> p h w", w=1))

            qt_sb = [qkpool.tile([128, T], F32R, tag=f"qt{g}", name=f"qt{g}")
                     for g in range(NG)]
            kt_sb = [qkpool.tile([128, T], F32R, tag=f"kt{g}", name=f"kt{g}")
                     for g in range(NG)]

            # ---------------- projection + RoPE ----------------
            for tch in range(T // TCH):
                t0 = tch * TCH
                xtc = xpool.tile([128, 8, TCH], F32R, tag="xt")
                nc.sync.dma_start(
                    xtc[:],
                    xt.rearrange("(a p) t -> p a t", p=128)[:, :, t0:t0 + TCH])

                # v projection: out[t, f] for all 8 heads
                for ts in range(TCH // 128):
                    vps = projps.tile([128, 512], F32, tag="proj")
                    for a in range(8):
                        nc.tensor.matmul(
                            vps[:], xtc[:, a, ts * 128:(ts + 1) * 128],
                            wv_sb[:, a, :], start=(a == 0), stop=(a == 7))
                    va = v_aug[tch * 4 + ts].rearrange(
                        "p (h w) -> p h w", w=65)
                    nc.vector.tensor_copy(
                        va[:, :, 0:64],
                        vps.rearrange("p (h d) -> p h d", d=64))

                # q/k projections per head-pair group + RoPE
                for g in range(NG):
                    for kind, wsb, dst in (("q", wq_sb, qt_sb[g]),
                                           ("k", wk_sb, kt_sb[g])):
                        ps = projps.tile([128, TCH], F32, tag="proj")
                        for a in range(8):
                            nc.tensor.matmul(
                                ps[:], wsb[:, a, g * 128:(g + 1) * 128],
                                xtc[:, a, :], start=(a == 0), stop=(a == 7))
                        # RoPE in permuted-d layout: rotate-half pairs sit
                        # 16 partitions apart within a 32-quadrant, so
                        # stream_shuffle provides the rotated operand and
                        # all ops are full-width contiguous.
                        rq = rpool.tile([128, TCH], F32, tag="rq")
                        t1 = rpool.tile([128, TCH], F32, tag="t1")
                        t2 = rpool.tile([128, TCH], F32, tag="t2")
                        nc.vector.stream_shuffle(rq[:], ps[:], SHUF_MASK)
                        nc.vector.tensor_mul(
                            t1[:], ps[:], ct_sb[:, t0:t0 + TCH])
                        nc.vector.tensor_mul(
                            t2[:], rq[:], st_sb[:, t0:t0 + TCH])
                        nc.vector.tensor_add(
                            dst[:, t0:t0 + TCH], t1[:], t2[:])

                # ---- attention for q-block qb == tch (k/v/q ready) ----
                qb = tch
                for g in range(NG):
                    q0 = qb * 512
                    nkb = 4 * (qb + 1)
                    out_t = [outps.tile([65, 512], F32, tag="outt", name=f"ot{g}_{qb}_{i}")
                             for i in range(2)]
                    for kbp in range(nkb // 2):
                        sc = [scoreps.tile([128, 1024], F32, tag="score", name=f"sc{g}_{qb}_{kbp}_{i}")
                              for i in range(2)]
                        pt = [ppool.tile([128, 1024], F32R, tag="pt", name=f"pt{g}_{qb}_{kbp}_{i}")
                              for i in range(2)]
                        for half in range(2):
                            kb = kbp * 2 + half
                            for h in range(2):
                                nc.tensor.matmul(
                                    sc[h][:, half * 512:(half + 1) * 512],
                                    kt_sb[g][h * 64:(h + 1) * 64,
                                             kb * 128:(kb + 1) * 128],
                                    qt_sb[g][h * 64:(h + 1) * 64,
                                             q0:q0 + 512],
                                    start=True, stop=True)
                        for half in range(2):
                            kb = kbp * 2 + half
                            off = max(0, (kb - 4 * qb)) * 128
                            co = half * 512
                            for h in range(2):
                                nc.scalar.activation(
                                    pt[h][:, co + off:co + 512],
                                    sc[h][:, co + off:co + 512],
                                    mybir.ActivationFunctionType.Exp,
                                    scale=0.125)
                                if kb >= 4 * qb:
                                    nc.vector.tensor_mul(
                                        pt[h][:, co + off:co + off + 128],
                                        pt[h][:, co + off:co + off + 128],
                                        tri_sb[:])
                                gh = 2 * g + h
                                nc.tensor.matmul(
                                    out_t[h][:, off:512],
                                    v_aug[kb][:, 65 * gh:65 * gh + 65],
                                    pt[h][:, co + off:co + 512],
                                    start=(kb == 0), stop=(kb == nkb - 1),
                                    skip_group_check=True)
                    for h in range(2):
                        gh = 2 * g + h
                        rec = npool.tile([1, 512], F32, tag="rec")
                        bc = npool.tile([64, 512], F32, tag="bc")
                        aot = aopool.tile([64, 512], F32R, tag="aot")
                        nc.vector.reciprocal(rec[:], out_t[h][64:65, :])
                        nc.gpsimd.partition_broadcast(bc[:], rec[:])
                        nc.vector.tensor_mul(aot[:], out_t[h][0:64, :], bc[:])
                        nc.sync.dma_start(
                            ao[gh * 64:(gh + 1) * 64, q0:q0 + 512], aot[:])
    nc.compile()
    return nc


def build_launch2():
    nc = bacc.Bacc("TRN2", target_bir_lowering=False, debug=False,
                   num_devices=NCORE)
    ao2 = nc.dram_tensor("ao2", [D, 1024], F32R, kind="ExternalInput")
    wo = nc.dram_tensor("wo", [D, D], F32R, kind="ExternalInput")
    y = nc.dram_tensor("y", [D, 1024], F32, kind="ExternalOutput")

    with tile.TileContext(nc) as tc:
        with (
            tc.tile_pool(name="asb", bufs=1) as apool,
            tc.tile_pool(name="wsb", bufs=2) as wpool,
            tc.tile_pool(name="ysb", bufs=3) as ypool,
            tc.tile_pool(name="ps", bufs=4, space="PSUM") as psp,
        ):
            a_sb = apool.tile([128, 8, 1024], F32R, tag="a")
            nc.sync.dma_start(
                a_sb[:], ao2.rearrange("(a p) t -> p a t", p=128))
            for ot in range(8):
                wt = wpool.tile([128, 8, 128], F32R, tag="w")
                nc.sync.dma_start(
                    wt[:], wo.rearrange("(a p) o -> p a o", p=128)
                    [:, :, ot * 128:(ot + 1) * 128])
                for tt in range(2):
                    ps = psp.tile([128, 512], F32, tag="ps")
                    for a in range(8):
                        nc.tensor.matmul(
                            ps[:], wt[:, a, :],
                            a_sb[:, a, tt * 512:(tt + 1) * 512],
                            start=(a == 0), stop=(a == 7))
                    ysb = ypool.tile([128, 512], F32, tag="y")
                    nc.vector.tensor_copy(ysb[:], ps[:])
                    nc.sync.dma_start(
                        y[ot * 128:(ot + 1) * 128,
                          tt * 512:(tt + 1) * 512], ysb[:])
    nc.compile()
    return nc


_NC1 = None
_NC2 = None
LAST_EXEC1_NS = None
LAST_EXEC2_NS = None


def kernel(x, cos, sin, w_qkv, w_out):
    global _NC1, _NC2
    x = np.asarray(x, dtype=np.float32)
    cos = np.asarray(cos, dtype=np.float32)
    sin = np.asarray(sin, dtype=np.float32)
    w_qkv = np.asarray(w_qkv, dtype=np.float32)
    w_out = np.asarray(w_out, dtype=np.float32)
    if _NC1 is None:
        _NC1 = build_launch1()
        _NC2 = build_launch2()

    perm = np.asarray(PERM)
    sign = np.where(perm < 32, -1.0, 1.0).astype(np.float32)
    ctp = cos.T[perm]                          # [64, T] permuted-d cos
    stp = sin.T[perm] * sign[:, None]          # signed, permuted sin
    ctm = np.ascontiguousarray(np.vstack([ctp, ctp]))  # [128, T] head pair
    stm = np.ascontiguousarray(np.vstack([stp, stp]))
    trim = np.ascontiguousarray(
        (np.arange(128)[:, None] <= np.arange(128)[None, :])
        .astype(np.float32))                   # tri[k, q] = k <= q
    wqm = w_qkv[0:D]                           # [f, c] head-major
    wkm = w_qkv[D:2 * D]
    wvm = w_qkv[2 * D:3 * D]
    # permute rotate-half pairs within each head's 64 q/k channels
    wqp = wqm.reshape(H, HD, D)[:, perm].reshape(D, D)
    wkp = wkm.reshape(H, HD, D)[:, perm].reshape(D, D)
    ones8 = np.ones((128, 8), np.float32)

    ins1 = []
    for core in range(NCORE):
        b, hh = core // 2, core % 2
        r0 = hh * 512
        ins1.append({
            "xt": np.ascontiguousarray(x[b].T),
            "wq": np.ascontiguousarray(wqp[r0:r0 + 512].T),
            "wk": np.ascontiguousarray(wkp[r0:r0 + 512].T),
            "wv": np.ascontiguousarray(wvm[r0:r0 + 512].T),
            "ct": ctm, "st": stm, "tri": trim, "vones": ones8,
        })
    res1 = run_bass_kernel_spmd(_NC1, ins1, core_ids=list(range(NCORE)))
    global LAST_EXEC1_NS, LAST_EXEC2_NS
    LAST_EXEC1_NS = res1.exec_time_ns

    wom = np.ascontiguousarray(w_out.T)        # [c, o]
    ins2 = []
    for core in range(NCORE):
        b, th = core // 2, core % 2
        aot = np.vstack([res1.results[2 * b]["ao"],
                         res1.results[2 * b + 1]["ao"]])  # [1024 c, 2048 t]
        ins2.append({
            "ao2": np.ascontiguousarray(aot[:, th * 1024:(th + 1) * 1024]),
            "wo": wom,
        })
    res2 = run_bass_kernel_spmd(_NC2, ins2, core_ids=list(range(NCORE)))
    LAST_EXEC2_NS = res2.exec_time_ns

    out = np.empty((B, T, D), dtype=np.float32)
    for core in range(NCORE):
        b, th = core // 2, core % 2
        out[b, th * 1024:(th + 1) * 1024, :] = res2.results[core]["y"].T
    return out


# revision 30
# speedup vs baseline: 1.2543x; 1.2462x over previous
"""Causal MHA + RoPE kernel for 8 trn2 NeuronCores.

Sharding: core id = (b, hh) with b = id//2 (batch), hh = id%2 (head half).
Launch 1: each core computes qkv projection + RoPE + causal attention for
its 8 heads over the full sequence of its batch element, writing the
(normalized) attention output slice ao [512 c, 2048 t] (channel-major).
Host: stacks the two head-half slices per batch -> aoT [1024 c, 2048 t].
Launch 2: core id = (b, th): output projection for a 1024-row t-slab.
All matmuls run in float32r (full PE rate at free-dim >= 256).
"""

import numpy as np
import concourse.bass as bass
import concourse.bacc as bacc
import concourse.mybir as mybir
from concourse import tile
from concourse.bass_utils import run_bass_kernel_spmd

B, T, D, H, HD = 4, 2048, 1024, 16, 64
F32 = mybir.dt.float32
F32R = mybir.dt.float32r
NCORE = 8
NG = 4          # head-pair groups per core (8 heads / 2)
TCH = 512       # t-chunk width in projection
NKB = T // 128  # 16 k-blocks
NQB = T // 512  # 4 q-blocks
# head-dim permutation putting rotate-half pairs 16 partitions apart
PERM = (list(range(0, 16)) + list(range(32, 48))
        + list(range(16, 32)) + list(range(48, 64)))
SHUF_MASK = [(i + 16) % 32 for i in range(32)]


def build_launch1():
    nc = bacc.Bacc("TRN2", target_bir_lowering=False, debug=False,
                   num_devices=NCORE)
    xt = nc.dram_tensor("xt", [D, T], F32R, kind="ExternalInput")
    wq = nc.dram_tensor("wq", [D, 512], F32R, kind="ExternalInput")
    wk = nc.dram_tensor("wk", [D, 512], F32R, kind="ExternalInput")
    wv = nc.dram_tensor("wv", [D, 512], F32R, kind="ExternalInput")
    ct = nc.dram_tensor("ct", [128, T], F32, kind="ExternalInput")
    st = nc.dram_tensor("st", [128, T], F32, kind="ExternalInput")
    tri = nc.dram_tensor("tri", [128, 128], F32R, kind="ExternalInput")
    vones = nc.dram_tensor("vones", [128, 8], F32R, kind="ExternalInput")
    ao = nc.dram_tensor("ao", [512, T], F32R, kind="ExternalOutput")
    dbg = nc.dram_tensor("dbg", [1, 64], F32, kind="ExternalOutput")

    with tile.TileContext(nc) as tc:
        with (
            tc.tile_pool(name="const", bufs=1) as cpool,
            tc.tile_pool(name="wsb", bufs=1) as wpool,
            tc.tile_pool(name="xtc", bufs=3) as xpool,
            tc.tile_pool(name="vaug", bufs=1) as vpool,
            tc.tile_pool(name="qkt", bufs=1) as qkpool,
            tc.tile_pool(name="qtp", bufs=2) as qpool,
            tc.tile_pool(name="rope", bufs=1) as rpool,
            tc.tile_pool(name="pt", bufs=2) as ppool,
            tc.tile_pool(name="norm", bufs=1) as npool,
            tc.tile_pool(name="aot", bufs=2) as aopool,
            tc.tile_pool(name="proj", bufs=2, space="PSUM") as projps,
            tc.tile_pool(name="score", bufs=2, space="PSUM") as scoreps,
            tc.tile_pool(name="outt", bufs=2, space="PSUM") as outps,
        ):
            ct_sb = cpool.tile([128, T], F32, tag="ct")
            st_sb = cpool.tile([128, T], F32, tag="st")
            vones_sb = cpool.tile([128, 8], F32R, tag="vones")
            nc.sync.dma_start(vones_sb[:], vones[:])
            tri_sb = cpool.tile([128, 128], F32R, tag="tri")
            nc.sync.dma_start(ct_sb[:], ct[:])
            nc.sync.dma_start(st_sb[:], st[:])
            nc.sync.dma_start(tri_sb[:], tri[:])

            wq_sb = wpool.tile([128, 8, 512], F32R, tag="wq")
            wk_sb = wpool.tile([128, 8, 512], F32R, tag="wk")
            wv_sb = wpool.tile([128, 8, 512], F32R, tag="wv")
            for wsb, wdr in ((wq_sb, wq), (wk_sb, wk), (wv_sb, wv)):
                nc.sync.dma_start(
                    wsb[:], wdr.rearrange("(a p) f -> p a f", p=128))

            # v_aug tiles: [128 t, 8 heads * 65]; col 65h+64 is the ones
            # column used to accumulate softmax denominators during AV.
            v_aug = [vpool.tile([128, 8 * 65], F32R, tag=f"va{i}", name=f"va{i}")
                     for i in range(NKB)]
            for i in range(NKB):
                va = v_aug[i].rearrange("p (h w) -> p h w", w=65)
                nc.sync.dma_start(
                    va[:, :, 64:65],
                    vones.rearrange("p (h w) -> p h w", w=1))

            kt_sb = [qkpool.tile([128, T], F32R, tag=f"kt{g}", name=f"kt{g}")
                     for g in range(NG)]
            qt_sb = [None] * NG

            # warm the PE HAM with a burst of bf16 matmuls so the f32r
            # stream that follows runs at full clock
            xb = cpool.tile([128, 512], mybir.dt.bfloat16, tag="xb")
            nc.vector.memset(xb[:], 0.125)
            wdbg = cpool.tile([1, 64], F32, tag="wdbg")
            for i in range(24):
                wps = projps.tile([128, 512], F32, tag="proj",
                                  name=f"wps{i}")
                nc.tensor.matmul(wps[:], xb[:, 0:128], xb[:],
                                 start=True, stop=True)
                if i == 23:
                    nc.vector.tensor_copy(wdbg[:], wps[0:1, 0:64])
            nc.sync.dma_start(dbg[:], wdbg[:])

            # ---------------- projection + RoPE ----------------
            for tch in range(T // TCH):
                t0 = tch * TCH
                xth = []
                for hf in range(2):
                    xc = xpool.tile([128, 8, TCH // 2], F32R, tag="xt",
                                    name=f"xt{tch}_{hf}")
                    nc.sync.dma_start(
                        xc[:],
                        xt.rearrange("(a p) t -> p a t", p=128)
                        [:, :, t0 + hf * 256:t0 + (hf + 1) * 256])
                    xth.append(xc)

                # v projection: out[t, f] for all 8 heads
                for ts in range(TCH // 128):
                    xc = xth[ts // 2]
                    tso = (ts % 2) * 128
                    vps = projps.tile([128, 512], F32, tag="proj")
                    for a in range(8):
                        nc.tensor.matmul(
                            vps[:], xc[:, a, tso:tso + 128],
                            wv_sb[:, a, :], start=(a == 0), stop=(a == 7))
                    va = v_aug[tch * 4 + ts].rearrange(
                        "p (h w) -> p h w", w=65)
                    nc.vector.tensor_copy(
                        va[:, :, 0:64],
                        vps.rearrange("p (h d) -> p h d", d=64))

                # q/k projections per head-pair group + RoPE
                for g in range(NG):
                    qt_g = qpool.tile([128, TCH], F32R, tag=f"qt{g}",
                                      name=f"qt{g}_{tch}")
                    qt_sb[g] = qt_g
                    for kind, wsb, dst, dof in (
                            ("q", wq_sb, qt_g, -t0),
                            ("k", wk_sb, kt_sb[g], 0)):
                        ps = projps.tile([128, TCH], F32, tag="proj")
                        for hf in range(2):
                            for a in range(8):
                                nc.tensor.matmul(
                                    ps[:, hf * 256:(hf + 1) * 256],
                                    wsb[:, a, g * 128:(g + 1) * 128],
                                    xth[hf][:, a, :],
                                    start=(a == 0), stop=(a == 7))
                        # RoPE in permuted-d layout: rotate-half pairs sit
                        # 16 partitions apart within a 32-quadrant, so
                        # stream_shuffle provides the rotated operand and
                        # all ops are full-width contiguous.
                        rq = rpool.tile([128, TCH], F32, tag="rq")
                        t1 = rpool.tile([128, TCH], F32, tag="t1")
                        nc.vector.stream_shuffle(rq[:], ps[:], SHUF_MASK)
                        nc.vector.tensor_mul(
                            t1[:], ps[:], ct_sb[:, t0:t0 + TCH])
                        nc.vector.tensor_mul(
                            rq[:], rq[:], st_sb[:, t0:t0 + TCH])
                        nc.vector.tensor_add(
                            dst[:, t0 + dof:t0 + dof + TCH], t1[:], rq[:])

                # ---- attention for q-block qb == tch (k/v/q ready) ----
                qb = tch
                for g in range(NG):
                    q0 = qb * 512
                    nkb = 4 * (qb + 1)
                    out_t = [outps.tile([65, 512], F32, tag="outt", name=f"ot{g}_{qb}_{i}")
                             for i in range(2)]
                    for kbp in range(nkb // 2):
                        sc = [scoreps.tile([128, 1024], F32, tag="score", name=f"sc{g}_{qb}_{kbp}_{i}")
                              for i in range(2)]
                        pt = [ppool.tile([128, 1024], F32R, tag="pt", name=f"pt{g}_{qb}_{kbp}_{i}")
                              for i in range(2)]
                        for half in range(2):
                            kb = kbp * 2 + half
                            for h in range(2):
                                nc.tensor.matmul(
                                    sc[h][:, half * 512:(half + 1) * 512],
                                    kt_sb[g][h * 64:(h + 1) * 64,
                                             kb * 128:(kb + 1) * 128],
                                    qt_sb[g][h * 64:(h + 1) * 64, 0:512],
                                    start=True, stop=True)
                        for half in range(2):
                            kb = kbp * 2 + half
                            off = max(0, (kb - 4 * qb)) * 128
                            co = half * 512
                            for h in range(2):
                                nc.scalar.activation(
                                    pt[h][:, co + off:co + 512],
                                    sc[h][:, co + off:co + 512],
                                    mybir.ActivationFunctionType.Exp,
                                    scale=0.125)
                                if kb >= 4 * qb:
                                    nc.vector.tensor_mul(
                                        pt[h][:, co + off:co + off + 128],
                                        pt[h][:, co + off:co + off + 128],
                                        tri_sb[:])
                                gh = 2 * g + h
                                nc.tensor.matmul(
                                    out_t[h][:, off:512],
                                    v_aug[kb][:, 65 * gh:65 * gh + 65],
                                    pt[h][:, co + off:co + 512],
                                    start=(kb == 0), stop=(kb == nkb - 1),
                                    skip_group_check=True)
                    for h in range(2):
                        gh = 2 * g + h
                        den = npool.tile([1, 512], F32, tag="den")
                        rec = npool.tile([1, 512], F32, tag="rec")
                        bc = npool.tile([64, 512], F32, tag="bc")
                        aot = aopool.tile([64, 512], F32R, tag="aot")
                        nc.vector.tensor_copy(den[:], out_t[h][64:65, :])
                        nc.vector.reciprocal_approx_fast(rec[:], den[:])
                        nc.gpsimd.partition_broadcast(bc[:], rec[:])
                        nc.vector.tensor_mul(aot[:], out_t[h][0:64, :], bc[:])
                        nc.sync.dma_start(
                            ao[gh * 64:(gh + 1) * 64, q0:q0 + 512], aot[:])
    nc.compile()
    return nc


def build_launch2():
    nc = bacc.Bacc("TRN2", target_bir_lowering=False, debug=False,
                   num_devices=NCORE)
    ao2 = nc.dram_tensor("ao2", [D, 1024], F32R, kind="ExternalInput")
    wo = nc.dram_tensor("wo", [D, D], F32R, kind="ExternalInput")
    y = nc.dram_tensor("y", [D, 1024], F32, kind="ExternalOutput")
    dbg = nc.dram_tensor("dbg", [1, 64], F32, kind="ExternalOutput")

    with tile.TileContext(nc) as tc:
        with (
            tc.tile_pool(name="asb", bufs=1) as apool,
            tc.tile_pool(name="wsb", bufs=2) as wpool,
            tc.tile_pool(name="ysb", bufs=3) as ypool,
            tc.tile_pool(name="ps", bufs=4, space="PSUM") as psp,
        ):
            xb = apool.tile([128, 512], mybir.dt.bfloat16, tag="xb")
            nc.vector.memset(xb[:], 0.125)
            wdbg = apool.tile([1, 64], F32, tag="wdbg")
            for i in range(20):
                wps = psp.tile([128, 512], F32, tag="ps", name=f"wps{i}")
                nc.tensor.matmul(wps[:], xb[:, 0:128], xb[:],
                                 start=True, stop=True)
                if i == 19:
                    nc.vector.tensor_copy(wdbg[:], wps[0:1, 0:64])
            nc.sync.dma_start(dbg[:], wdbg[:])
            a_sb = apool.tile([128, 8, 1024], F32R, tag="a")
            for a in range(8):
                nc.sync.dma_start(
                    a_sb[:, a, :],
                    ao2.rearrange("(a p) t -> p a t", p=128)[:, a, :])
            for ot in range(8):
                wt = wpool.tile([128, 8, 128], F32R, tag="w")
                nc.sync.dma_start(
                    wt[:], wo.rearrange("(a p) o -> p a o", p=128)
                    [:, :, ot * 128:(ot + 1) * 128])
                for tt in range(2):
                    ps = psp.tile([128, 512], F32, tag="ps")
                    for a in range(8):
                        nc.tensor.matmul(
                            ps[:], wt[:, a, :],
                            a_sb[:, a, tt * 512:(tt + 1) * 512],
                            start=(a == 0), stop=(a == 7))
                    ysb = ypool.tile([128, 512], F32, tag="y")
                    nc.vector.tensor_copy(ysb[:], ps[:])
                    nc.sync.dma_start(
                        y[ot * 128:(ot + 1) * 128,
                          tt * 512:(tt + 1) * 512], ysb[:])
    nc.compile()
    return nc


_NC1 = None
_NC2 = None
LAST_EXEC1_NS = None
LAST_EXEC2_NS = None


def kernel(x, cos, sin, w_qkv, w_out):
    global _NC1, _NC2
    x = np.asarray(x, dtype=np.float32)
    cos = np.asarray(cos, dtype=np.float32)
    sin = np.asarray(sin, dtype=np.float32)
    w_qkv = np.asarray(w_qkv, dtype=np.float32)
    w_out = np.asarray(w_out, dtype=np.float32)
    if _NC1 is None:
        _NC1 = build_launch1()
        _NC2 = build_launch2()

    perm = np.asarray(PERM)
    sign = np.where(perm < 32, -1.0, 1.0).astype(np.float32)
    ctp = cos.T[perm]                          # [64, T] permuted-d cos
    stp = sin.T[perm] * sign[:, None]          # signed, permuted sin
    ctm = np.ascontiguousarray(np.vstack([ctp, ctp]))  # [128, T] head pair
    stm = np.ascontiguousarray(np.vstack([stp, stp]))
    trim = np.ascontiguousarray(
        (np.arange(128)[:, None] <= np.arange(128)[None, :])
        .astype(np.float32))                   # tri[k, q] = k <= q
    wqm = w_qkv[0:D]                           # [f, c] head-major
    wkm = w_qkv[D:2 * D]
    wvm = w_qkv[2 * D:3 * D]
    # permute rotate-half pairs within each head's 64 q/k channels
    wqp = wqm.reshape(H, HD, D)[:, perm].reshape(D, D)
    wkp = wkm.reshape(H, HD, D)[:, perm].reshape(D, D)
    ones8 = np.ones((128, 8), np.float32)

    ins1 = []
    for core in range(NCORE):
        b, hh = core // 2, core % 2
        r0 = hh * 512
        ins1.append({
            "xt": np.ascontiguousarray(x[b].T),
            "wq": np.ascontiguousarray(wqp[r0:r0 + 512].T),
            "wk": np.ascontiguousarray(wkp[r0:r0 + 512].T),
            "wv": np.ascontiguousarray(wvm[r0:r0 + 512].T),
            "ct": ctm, "st": stm, "tri": trim, "vones": ones8,
        })
    res1 = run_bass_kernel_spmd(_NC1, ins1, core_ids=list(range(NCORE)))
    global LAST_EXEC1_NS, LAST_EXEC2_NS
    LAST_EXEC1_NS = res1.exec_time_ns

    wom = np.ascontiguousarray(w_out.T)        # [c, o]
    ins2 = []
    for core in range(NCORE):
        b, th = core // 2, core % 2
        aot = np.vstack([res1.results[2 * b]["ao"],
                         res1.results[2 * b + 1]["ao"]])  # [1024 c, 2048 t]
        ins2.append({
            "ao2": np.ascontiguousarray(aot[:, th * 1024:(th + 1) * 1024]),
            "wo": wom,
        })
    res2 = run_bass_kernel_spmd(_NC2, ins2, core_ids=list(range(NCORE)))
    LAST_EXEC2_NS = res2.exec_time_ns

    out = np.empty((B, T, D), dtype=np.float32)
    for core in range(NCORE):
        b, th = core // 2, core % 2
        out[b, th * 1024:(th + 1) * 1024, :] = res2.results[core]["y"].T
    return out


# revision 32
# speedup vs baseline: 1.2567x; 1.0019x over previous
"""Causal MHA + RoPE kernel for 8 trn2 NeuronCores.

Sharding: core id = (b, hh) with b = id//2 (batch), hh = id%2 (head half).
Launch 1: each core computes qkv projection + RoPE + causal attention for
its 8 heads over the full sequence of its batch element, writing the
(normalized) attention output slice ao [512 c, 2048 t] (channel-major).
Host: stacks the two head-half slices per batch -> aoT [1024 c, 2048 t].
Launch 2: core id = (b, th): output projection for a 1024-row t-slab.
All matmuls run in float32r (full PE rate at free-dim >= 256).
"""

import numpy as np
import concourse.bass as bass
import concourse.bacc as bacc
import concourse.mybir as mybir
from concourse import tile
from concourse.bass_utils import run_bass_kernel_spmd

B, T, D, H, HD = 4, 2048, 1024, 16, 64
F32 = mybir.dt.float32
F32R = mybir.dt.float32r
NCORE = 8
NG = 4          # head-pair groups per core (8 heads / 2)
TCH = 512       # t-chunk width in projection
NKB = T // 128  # 16 k-blocks
NQB = T // 512  # 4 q-blocks
# head-dim permutation putting rotate-half pairs 16 partitions apart
PERM = (list(range(0, 16)) + list(range(32, 48))
        + list(range(16, 32)) + list(range(48, 64)))
SHUF_MASK = [(i + 16) % 32 for i in range(32)]


def build_launch1():
    nc = bacc.Bacc("TRN2", target_bir_lowering=False, debug=False,
                   num_devices=NCORE)
    xt = nc.dram_tensor("xt", [D, T], F32R, kind="ExternalInput")
    wq = nc.dram_tensor("wq", [D, 512], F32R, kind="ExternalInput")
    wk = nc.dram_tensor("wk", [D, 512], F32R, kind="ExternalInput")
    wv = nc.dram_tensor("wv", [D, 512], F32R, kind="ExternalInput")
    ct = nc.dram_tensor("ct", [128, T], F32, kind="ExternalInput")
    st = nc.dram_tensor("st", [128, T], F32, kind="ExternalInput")
    tri = nc.dram_tensor("tri", [128, 128], F32R, kind="ExternalInput")
    vones = nc.dram_tensor("vones", [128, 8], F32R, kind="ExternalInput")
    ao = nc.dram_tensor("ao", [512, T], F32R, kind="ExternalOutput")
    dbg = nc.dram_tensor("dbg", [1, 64], F32, kind="ExternalOutput")

    with tile.TileContext(nc) as tc:
        with (
            tc.tile_pool(name="const", bufs=1) as cpool,
            tc.tile_pool(name="wsb", bufs=1) as wpool,
            tc.tile_pool(name="xtc", bufs=3) as xpool,
            tc.tile_pool(name="vaug", bufs=1) as vpool,
            tc.tile_pool(name="qkt", bufs=1) as qkpool,
            tc.tile_pool(name="qtp", bufs=2) as qpool,
            tc.tile_pool(name="rope", bufs=1) as rpool,
            tc.tile_pool(name="pt", bufs=2) as ppool,
            tc.tile_pool(name="norm", bufs=1) as npool,
            tc.tile_pool(name="aot", bufs=2) as aopool,
            tc.tile_pool(name="proj", bufs=2, space="PSUM") as projps,
            tc.tile_pool(name="score", bufs=2, space="PSUM") as scoreps,
            tc.tile_pool(name="outt", bufs=2, space="PSUM") as outps,
        ):
            ct_sb = cpool.tile([128, T], F32, tag="ct")
            st_sb = cpool.tile([128, T], F32, tag="st")
            vones_sb = cpool.tile([128, 8], F32R, tag="vones")
            nc.sync.dma_start(vones_sb[:], vones[:])
            tri_sb = cpool.tile([128, 128], F32R, tag="tri")
            nc.sync.dma_start(ct_sb[:], ct[:])
            nc.sync.dma_start(st_sb[:], st[:])
            nc.sync.dma_start(tri_sb[:], tri[:])

            wq_sb = wpool.tile([128, 8, 512], F32R, tag="wq")
            wk_sb = wpool.tile([128, 8, 512], F32R, tag="wk")
            wv_sb = wpool.tile([128, 8, 512], F32R, tag="wv")
            for wsb, wdr in ((wq_sb, wq), (wk_sb, wk), (wv_sb, wv)):
                nc.sync.dma_start(
                    wsb[:], wdr.rearrange("(a p) f -> p a f", p=128))

            # v_aug tiles: [128 t, 8 heads * 65]; col 65h+64 is the ones
            # column used to accumulate softmax denominators during AV.
            v_aug = [vpool.tile([128, 8 * 65], F32R, tag=f"va{i}", name=f"va{i}")
                     for i in range(NKB)]
            for i in range(NKB):
                va = v_aug[i].rearrange("p (h w) -> p h w", w=65)
                nc.sync.dma_start(
                    va[:, :, 64:65],
                    vones.rearrange("p (h w) -> p h w", w=1))

            kt_sb = [qkpool.tile([128, T], F32R, tag=f"kt{g}", name=f"kt{g}")
                     for g in range(NG)]
            qt_sb = [None] * NG

            # warm the PE HAM with a burst of bf16 matmuls so the f32r
            # stream that follows runs at full clock
            xb = cpool.tile([128, 512], mybir.dt.bfloat16, tag="xb")
            nc.vector.memset(xb[:], 0.125)
            wdbg = cpool.tile([1, 64], F32, tag="wdbg")
            for i in range(24):
                wps = projps.tile([128, 512], F32, tag="proj",
                                  name=f"wps{i}")
                nc.tensor.matmul(wps[:], xb[:, 0:128], xb[:],
                                 start=True, stop=True)
                if i == 23:
                    nc.vector.tensor_copy(wdbg[:], wps[0:1, 0:64])
            nc.sync.dma_start(dbg[:], wdbg[:])

            # ---------------- projection + RoPE ----------------
            for tch in range(T // TCH):
                t0 = tch * TCH
                xth = []
                for hf in range(2):
                    xc = xpool.tile([128, 8, TCH // 2], F32R, tag="xt",
                                    name=f"xt{tch}_{hf}")
                    nc.sync.dma_start(
                        xc[:],
                        xt.rearrange("(a p) t -> p a t", p=128)
                        [:, :, t0 + hf * 256:t0 + (hf + 1) * 256])
                    xth.append(xc)

                # v projection: out[t, f] for all 8 heads
                for ts in range(TCH // 128):
                    xc = xth[ts // 2]
                    tso = (ts % 2) * 128
                    vps = projps.tile([128, 512], F32, tag="proj")
                    for a in range(8):
                        nc.tensor.matmul(
                            vps[:], xc[:, a, tso:tso + 128],
                            wv_sb[:, a, :], start=(a == 0), stop=(a == 7))
                    va = v_aug[tch * 4 + ts].rearrange(
                        "p (h w) -> p h w", w=65)
                    nc.vector.tensor_copy(
                        va[:, :, 0:64],
                        vps.rearrange("p (h d) -> p h d", d=64))

                # q/k projections per head-pair group + RoPE
                for g in range(NG):
                    qt_g = qpool.tile([128, TCH], F32R, tag=f"qt{g}",
                                      name=f"qt{g}_{tch}")
                    qt_sb[g] = qt_g
                    for kind, wsb, dst, dof in (
                            ("q", wq_sb, qt_g, -t0),
                            ("k", wk_sb, kt_sb[g], 0)):
                        ps = projps.tile([128, TCH], F32, tag="proj")
                        for hf in range(2):
                            for a in range(8):
                                nc.tensor.matmul(
                                    ps[:, hf * 256:(hf + 1) * 256],
                                    wsb[:, a, g * 128:(g + 1) * 128],
                                    xth[hf][:, a, :],
                                    start=(a == 0), stop=(a == 7))
                        # RoPE in permuted-d layout: rotate-half pairs sit
                        # 16 partitions apart within a 32-quadrant, so
                        # stream_shuffle provides the rotated operand and
                        # all ops are full-width contiguous.
                        rq = rpool.tile([128, TCH], F32, tag="rq")
                        t1 = rpool.tile([128, TCH], F32, tag="t1")
                        nc.vector.stream_shuffle(rq[:], ps[:], SHUF_MASK)
                        nc.vector.tensor_mul(
                            t1[:], ps[:], ct_sb[:, t0:t0 + TCH])
                        nc.vector.tensor_mul(
                            rq[:], rq[:], st_sb[:, t0:t0 + TCH])
                        nc.vector.tensor_add(
                            dst[:, t0 + dof:t0 + dof + TCH], t1[:], rq[:])

                # ---- attention for q-block qb == tch (k/v/q ready) ----
                qb = tch
                for g in range(NG):
                    q0 = qb * 512
                    nkb = 4 * (qb + 1)
                    out_t = [outps.tile([65, 512], F32, tag="outt", name=f"ot{g}_{qb}_{i}")
                             for i in range(2)]
                    for kbp in range(nkb // 2):
                        sc = [scoreps.tile([128, 1024], F32, tag="score", name=f"sc{g}_{qb}_{kbp}_{i}")
                              for i in range(2)]
                        pt = [ppool.tile([128, 1024], F32R, tag="pt", name=f"pt{g}_{qb}_{kbp}_{i}")
                              for i in range(2)]
                        if kbp % 2 == 0:
                            # bf16 no-op matmul keeps the PE clock gate
                            # open (overwritten by the start=True QK)
                            nc.tensor.matmul(sc[0][:, 0:512],
                                             xb[:, 0:128], xb[:],
                                             start=True, stop=True)
                        for half in range(2):
                            kb = kbp * 2 + half
                            for h in range(2):
                                nc.tensor.matmul(
                                    sc[h][:, half * 512:(half + 1) * 512],
                                    kt_sb[g][h * 64:(h + 1) * 64,
                                             kb * 128:(kb + 1) * 128],
                                    qt_sb[g][h * 64:(h + 1) * 64, 0:512],
                                    start=True, stop=True)
                        for half in range(2):
                            kb = kbp * 2 + half
                            off = max(0, (kb - 4 * qb)) * 128
                            co = half * 512
                            for h in range(2):
                                nc.scalar.activation(
                                    pt[h][:, co + off:co + 512],
                                    sc[h][:, co + off:co + 512],
                                    mybir.ActivationFunctionType.Exp,
                                    scale=0.125)
                                if kb >= 4 * qb:
                                    nc.vector.tensor_mul(
                                        pt[h][:, co + off:co + off + 128],
                                        pt[h][:, co + off:co + off + 128],
                                        tri_sb[:])
                                gh = 2 * g + h
                                nc.tensor.matmul(
                                    out_t[h][:, off:512],
                                    v_aug[kb][:, 65 * gh:65 * gh + 65],
                                    pt[h][:, co + off:co + 512],
                                    start=(kb == 0), stop=(kb == nkb - 1),
                                    skip_group_check=True)
                    for h in range(2):
                        gh = 2 * g + h
                        den = npool.tile([1, 512], F32, tag="den")
                        rec = npool.tile([1, 512], F32, tag="rec")
                        bc = npool.tile([64, 512], F32, tag="bc")
                        aot = aopool.tile([64, 512], F32R, tag="aot")
                        nc.vector.tensor_copy(den[:], out_t[h][64:65, :])
                        nc.vector.reciprocal_approx_fast(rec[:], den[:])
                        nc.gpsimd.partition_broadcast(bc[:], rec[:])
                        nc.vector.tensor_mul(aot[:], out_t[h][0:64, :], bc[:])
                        nc.sync.dma_start(
                            ao[gh * 64:(gh + 1) * 64, q0:q0 + 512], aot[:])
    nc.compile()
    return nc


def build_launch2():
    nc = bacc.Bacc("TRN2", target_bir_lowering=False, debug=False,
                   num_devices=NCORE)
    ao2 = nc.dram_tensor("ao2", [D, 1024], F32R, kind="ExternalInput")
    wo = nc.dram_tensor("wo", [D, D], F32R, kind="ExternalInput")
    y = nc.dram_tensor("y", [D, 1024], F32, kind="ExternalOutput")
    dbg = nc.dram_tensor("dbg", [1, 64], F32, kind="ExternalOutput")

    with tile.TileContext(nc) as tc:
        with (
            tc.tile_pool(name="asb", bufs=1) as apool,
            tc.tile_pool(name="wsb", bufs=2) as wpool,
            tc.tile_pool(name="ysb", bufs=3) as ypool,
            tc.tile_pool(name="ps", bufs=4, space="PSUM") as psp,
        ):
            xb = apool.tile([128, 512], mybir.dt.bfloat16, tag="xb")
            nc.vector.memset(xb[:], 0.125)
            wdbg = apool.tile([1, 64], F32, tag="wdbg")
            for i in range(20):
                wps = psp.tile([128, 512], F32, tag="ps", name=f"wps{i}")
                nc.tensor.matmul(wps[:], xb[:, 0:128], xb[:],
                                 start=True, stop=True)
                if i == 19:
                    nc.vector.tensor_copy(wdbg[:], wps[0:1, 0:64])
            nc.sync.dma_start(dbg[:], wdbg[:])
            a_sb = apool.tile([128, 8, 1024], F32R, tag="a")
            for a in range(8):
                nc.sync.dma_start(
                    a_sb[:, a, :],
                    ao2.rearrange("(a p) t -> p a t", p=128)[:, a, :])
            for ot in range(8):
                wt = wpool.tile([128, 8, 128], F32R, tag="w")
                nc.sync.dma_start(
                    wt[:], wo.rearrange("(a p) o -> p a o", p=128)
                    [:, :, ot * 128:(ot + 1) * 128])
                for tt in range(2):
                    ps = psp.tile([128, 512], F32, tag="ps")
                    if tt == 0:
                        nc.tensor.matmul(ps[:], xb[:, 0:128], xb[:],
                                         start=True, stop=True)
                    for a in range(8):
                        nc.tensor.matmul(
                            ps[:], wt[:, a, :],
                            a_sb[:, a, tt * 512:(tt + 1) * 512],
                            start=(a == 0), stop=(a == 7))
                    ysb = ypool.tile([128, 512], F32, tag="y")
                    nc.vector.tensor_copy(ysb[:], ps[:])
                    nc.sync.dma_start(
                        y[ot * 128:(ot + 1) * 128,
                          tt * 512:(tt + 1) * 512], ysb[:])
    nc.compile()
    return nc


_NC1 = None
_NC2 = None
LAST_EXEC1_NS = None
LAST_EXEC2_NS = None


def kernel(x, cos, sin, w_qkv, w_out):
    global _NC1, _NC2
    x = np.asarray(x, dtype=np.float32)
    cos = np.asarray(cos, dtype=np.float32)
    sin = np.asarray(sin, dtype=np.float32)
    w_qkv = np.asarray(w_qkv, dtype=np.float32)
    w_out = np.asarray(w_out, dtype=np.float32)
    if _NC1 is None:
        _NC1 = build_launch1()
        _NC2 = build_launch2()

    perm = np.asarray(PERM)
    sign = np.where(perm < 32, -1.0, 1.0).astype(np.float32)
    ctp = cos.T[perm]                          # [64, T] permuted-d cos
    stp = sin.T[perm] * sign[:, None]          # signed, permuted sin
    ctm = np.ascontiguousarray(np.vstack([ctp, ctp]))  # [128, T] head pair
    stm = np.ascontiguousarray(np.vstack([stp, stp]))
    trim = np.ascontiguousarray(
        (np.arange(128)[:, None] <= np.arange(128)[None, :])
        .astype(np.float32))                   # tri[k, q] = k <= q
    wqm = w_qkv[0:D]                           # [f, c] head-major
    wkm = w_qkv[D:2 * D]
    wvm = w_qkv[2 * D:3 * D]
    # permute rotate-half pairs within each head's 64 q/k channels
    wqp = wqm.reshape(H, HD, D)[:, perm].reshape(D, D)
    wkp = wkm.reshape(H, HD, D)[:, perm].reshape(D, D)
    ones8 = np.ones((128, 8), np.float32)

    ins1 = []
    for core in range(NCORE):
        b, hh = core // 2, core % 2
        r0 = hh * 512
        ins1.append({
            "xt": np.ascontiguousarray(x[b].T),
            "wq": np.ascontiguousarray(wqp[r0:r0 + 512].T),
            "wk": np.ascontiguousarray(wkp[r0:r0 + 512].T),
            "wv": np.ascontiguousarray(wvm[r0:r0 + 512].T),
            "ct": ctm, "st": stm, "tri": trim, "vones": ones8,
        })
    res1 = run_bass_kernel_spmd(_NC1, ins1, core_ids=list(range(NCORE)))
    global LAST_EXEC1_NS, LAST_EXEC2_NS
    LAST_EXEC1_NS = res1.exec_time_ns

    wom = np.ascontiguousarray(w_out.T)        # [c, o]
    ins2 = []
    for core in range(NCORE):
        b, th = core // 2, core % 2
        aot = np.vstack([res1.results[2 * b]["ao"],
                         res1.results[2 * b + 1]["ao"]])  # [1024 c, 2048 t]
        ins2.append({
            "ao2": np.ascontiguousarray(aot[:, th * 1024:(th + 1) * 1024]),
            "wo": wom,
        })
    res2 = run_bass_kernel_spmd(_NC2, ins2, core_ids=list(range(NCORE)))
    LAST_EXEC2_NS = res2.exec_time_ns

    out = np.empty((B, T, D), dtype=np.float32)
    for core in range(NCORE):
        b, th = core // 2, core % 2
        out[b, th * 1024:(th + 1) * 1024, :] = res2.results[core]["y"].T
    return out
